# revision 11
# baseline (speedup 1.0000x reference)
"""Trainium2 Bass kernel for nn_CrossAttention: softmax(x Wq^T (x Wk^T)^T / sqrt(C)) @ (x Wv^T).

Sharding: data-parallel over batch B=8 across the 8 NeuronCores (one batch
element per core, no collectives).

Algebraic refactor: scores = x (Wq^T Wk) x^T, so M = Wq^T Wk is precomputed
on host (fp32, shared across cores) and the K projection disappears. Per
core: x^T lives SBUF-resident (loaded once), V = x Wv^T is projected into
resident SBUF with N=512 matmuls (s-major/hh-inner so V-proj consumes the
x stream in arrival order — the initial load is HBM-contended across the 8
cores), then per 512-token q-chunk: G^T = M^T x^T chunk projection (N=512),
S^T = x G^T (N=512), exp on ACT, PV with N=512 matmuls in two sequential
512-wide h-halves (4 PSUM banks each), softmax denominator via DVE adds +
ones-matmul reduction, recip-scale on DVE, DMA out in bf16 (host upcasts;
halves the output traffic and tail drain).

Measured: the 8-core steady state is power-throttled (~1.94GHz effective PE
clock vs 2.4GHz at 1 core — random-data toggle power trips the chip limit),
so the kernel runs within ~2% of that power roofline; remaining wins are in
the single-shot startup path (load pipelining), not the MM stream.
"""

import sys

sys.path.insert(0, "/opt/trn_rl_repo")

import numpy as np
import ml_dtypes

B, T, C, H = 8, 4096, 1024, 1024
NCORES = 8

CT = C // 128   # 8 contraction tiles
ST = T // 128   # 32 key tiles
QCH = 512       # attention q-chunk
NQCH = T // QCH
QS = QCH // 128  # q sub-tiles per chunk
XCH = 1024      # x_sb DMA chunk (cols)

_CACHE = {}


def _build(reps=1, loop=False):
    import concourse.bacc as bacc
    import concourse.tile as tile
    from concourse import mybir

    f32 = mybir.dt.float32
    bf16 = mybir.dt.bfloat16

    nc = bacc.Bacc("TRN2", target_bir_lowering=False, debug=False,
                   num_devices=NCORES)

    xT = nc.dram_tensor("xT", [C, T], bf16, kind="ExternalInput").ap()
    mT = nc.dram_tensor("mT", [C, C], bf16, kind="ExternalInput").ap()
    wvT = nc.dram_tensor("wvT", [C, H], bf16, kind="ExternalInput").ap()
    out = nc.dram_tensor("out", [T, H], bf16, kind="ExternalOutput").ap()

    # [c, t] -> [p, a, t] with c = a*128 + p
    xTr = xT.rearrange("(a p) t -> p a t", p=128)
    mr = mT.rearrange("(a p) h -> p a h", p=128)
    wvr = wvT.rearrange("(a p) h -> p a h", p=128)

    scale = 1.0 / np.sqrt(np.float32(C))

    with tile.TileContext(nc) as tc:
        with tc.tile_pool(name="singles", bufs=1) as singles, \
             tc.tile_pool(name="wsb", bufs=1) as wsb, \
             tc.tile_pool(name="qtp", bufs=1) as qtp, \
             tc.tile_pool(name="ptp", bufs=33) as ptp, \
             tc.tile_pool(name="accp", bufs=2) as accp, \
             tc.tile_pool(name="rcp", bufs=8) as rcp, \
             tc.tile_pool(name="op", bufs=2) as op, \
             tc.tile_pool(name="pss", bufs=3, space="PSUM") as pss, \
             tc.tile_pool(name="pso", bufs=5, space="PSUM") as pso:

            x_sb = singles.tile([128, CT, T], bf16, tag="x")      # x^T [c, t]
            v_sb = singles.tile([128, ST, H], bf16, tag="v")      # V [s, h]
            ones = singles.tile([128, 1], bf16, tag="ones")
            nc.vector.memset(ones, 1.0)

            def emit_rep(rep):
                # ---- Loads: wv (both halves) first, then the head of x so
                # V-proj can start, then M, then the x tail. V-proj is
                # s-major/hh-inner so it consumes x strictly in arrival
                # order — under 8-core HBM contention the x stream is the
                # long pole and V-proj pipelines with it instead of
                # re-sweeping x after the full load.
                wvA = wsb.tile([128, CT, 512], bf16, tag="wA",
                               name=f"wvA{rep}")
                wvB = wsb.tile([128, CT, 512], bf16, tag="wB",
                               name=f"wvB{rep}")
                wmA = wsb.tile([128, CT, 512], bf16, tag="wmA",
                               name=f"wmA{rep}")
                wmB = wsb.tile([128, CT, 512], bf16, tag="wmB",
                               name=f"wmB{rep}")
                # head loads split per c-slab so the first V-proj matmuls
                # start after ~one slab (0.5µs) instead of the full 1.25MB
                for c in range(CT):
                    nc.sync.dma_start(out=wvA[:, c, :], in_=wvr[:, c, 0:512])
                    nc.sync.dma_start(out=x_sb[:, c, 0:128],
                                      in_=xTr[:, c, 0:128])
                nc.sync.dma_start(out=wvB, in_=wvr[:, :, 512:1024])
                nc.sync.dma_start(out=x_sb[:, :, 128:XCH],
                                  in_=xTr[:, :, 128:XCH])
                for xch in range(1, T // XCH):
                    t0 = xch * XCH
                    nc.sync.dma_start(out=x_sb[:, :, t0:t0 + XCH],
                                      in_=xTr[:, :, t0:t0 + XCH])
                # M is first read by G-proj(chunk 0), which runs after the
                # whole V phase — load it after the x stream it would delay.
                nc.sync.dma_start(out=wmA, in_=mr[:, :, 0:512])
                nc.sync.dma_start(out=wmB, in_=mr[:, :, 512:1024])

                # ---- Phase 1: V = x @ Wv^T into resident SBUF ----
                wvh = [wvA, wvB]
                wmh = [wmA, wmB]
                for s in range(ST):
                    for hh in range(2):
                        ps = pso.tile([128, 512], f32, tag="o",
                                      name=f"psv{rep}_{s}_{hh}")
                        for c in range(CT):
                            nc.tensor.matmul(
                                ps,
                                x_sb[:, c, s * 128:(s + 1) * 128],
                                wvh[hh][:, c, :],
                                start=(c == 0), stop=(c == CT - 1))
                        nc.vector.tensor_copy(
                            out=v_sb[:, s, hh * 512:(hh + 1) * 512], in_=ps)

                # ---- Phase 2: attention, q-chunks of QCH ----
                def emit_gproj(qch):
                    # G^T[:, chunk] = M^T @ x^T[:, chunk]
                    q0 = qch * QCH
                    gt = qtp.tile([128, CT, QCH], bf16, tag="qt",
                                  name=f"gt{rep}_{qch}")
                    for co in range(CT):
                        wm = wmh[co // 4]
                        co4 = co % 4
                        ps = pss.tile([128, QCH], f32, tag="s",
                                      name=f"psq{rep}_{qch}_{co}")
                        for c in range(CT):
                            nc.tensor.matmul(ps,
                                             wm[:, c, co4 * 128:(co4 + 1) * 128],
                                             x_sb[:, c, q0:q0 + QCH],
                                             start=(c == 0), stop=(c == CT - 1))
                        nc.scalar.copy(out=gt[:, co, :], in_=ps)
                    return gt

                gt_next = emit_gproj(0)
                for qch in range(NQCH):
                    q0 = qch * QCH
                    gt = gt_next
                    # scores S^T[s, q] = x @ G^T ; P = exp(S * scale)
                    acc = accp.tile([128, QCH], f32, tag="acc",
                                    name=f"acc{rep}_{qch}")
                    pts = []
                    for s in range(ST):
                        ps = pss.tile([128, QCH], f32, tag="s",
                                      name=f"pss{rep}_{qch}_{s}")
                        for c in range(CT):
                            nc.tensor.matmul(ps,
                                             x_sb[:, c, s * 128:(s + 1) * 128],
                                             gt[:, c, :],
                                             start=(c == 0), stop=(c == CT - 1))
                        pt = ptp.tile([128, QCH], bf16, tag="pt",
                                      name=f"pt{rep}_{qch}_{s}")
                        nc.scalar.activation(out=pt, in_=ps,
                                             func=mybir.ActivationFunctionType.Exp,
                                             scale=float(scale))
                        pts.append(pt)
                        if s == 0:
                            nc.vector.tensor_copy(out=acc, in_=pt)
                        else:
                            nc.vector.tensor_add(out=acc, in0=acc, in1=pt)
                    def emit_recips():
                        # rowsum -> reciprocal; emitted after enough PE work
                        # that the DVE add chain into acc has finished.
                        # acc is rounded to bf16 first so the ones-matmuls
                        # use bf16 stationaries (pipelined FWL weight loads;
                        # an fp32 stationary self-loads serially and breaks
                        # the bf16 MM stream). Rounding 128 partials costs
                        # ~0.04% on the denominator.
                        accb = ptp.tile([128, QCH], bf16, tag="pt",
                                        name=f"accb{rep}_{qch}")
                        nc.vector.tensor_copy(out=accb, in_=acc)
                        rcs = []
                        for j in range(QS):
                            psr = pss.tile([128, 1], f32, tag="s",
                                           name=f"psr{rep}_{qch}_{j}")
                            nc.tensor.matmul(psr,
                                             accb[:, j * 128:(j + 1) * 128],
                                             ones, start=True, stop=True)
                            rc = rcp.tile([128, 1], f32, tag="rc",
                                          name=f"rc{rep}_{qch}_{j}")
                            nc.vector.reciprocal(out=rc, in_=psr)
                            rcs.append(rc)
                        return rcs

                    # hoisted G^T projection for the next chunk: its PE work
                    # lands between QK and PV so ACT copies overlap PV
                    recips = None
                    if qch + 1 < NQCH:
                        gt_next = emit_gproj(qch + 1)
                        recips = emit_recips()
                    # O = P^T.T @ V over all 32 s tiles, h in two 512-halves
                    for hh in range(2):
                        pos = [pso.tile([128, 512], f32, tag="o",
                                        name=f"po{rep}_{qch}_{hh}_{j}")
                               for j in range(QS)]
                        for s in range(ST):
                            for j in range(QS):
                                nc.tensor.matmul(
                                    pos[j],
                                    pts[s][:, j * 128:(j + 1) * 128],
                                    v_sb[:, s, hh * 512:(hh + 1) * 512],
                                    start=(s == 0),
                                    stop=(s == ST - 1),
                                    skip_group_check=True)
                        if recips is None:
                            recips = emit_recips()
                        for j in range(QS):
                            ob = op.tile([128, 512], bf16, tag="ob",
                                         name=f"ob{rep}_{qch}_{hh}_{j}")
                            nc.vector.tensor_scalar_mul(ob, pos[j], recips[j])
                            nc.sync.dma_start(
                                out=out[q0 + j * 128:q0 + (j + 1) * 128,
                                        hh * 512:(hh + 1) * 512],
                                in_=ob)

            if loop and reps > 1:
                from concourse import mybir as _mb
                engs = [_mb.EngineType.PE, _mb.EngineType.Activation,
                        _mb.EngineType.DVE, _mb.EngineType.SP]
                with tc.For_i(0, reps, 1, hint_engines=tuple(engs)):
                    emit_rep(0)
            else:
                for rep in range(reps):
                    emit_rep(rep)

    nc.compile()
    return nc


def _get_program(reps=1):
    if reps not in _CACHE:
        _CACHE[reps] = _build(reps)
    return _CACHE[reps]


def prep_inputs(x, Wq, Wk, Wv):
    """Host-side shard + layout prep: returns per-core input maps."""
    x = np.asarray(x, dtype=np.float32)
    bf = ml_dtypes.bfloat16
    M = (np.asarray(Wq, dtype=np.float32).T
         @ np.asarray(Wk, dtype=np.float32)).astype(bf)
    wvT = np.ascontiguousarray(np.asarray(Wv, dtype=np.float32).T).astype(bf)
    in_maps = []
    for b in range(NCORES):
        xTb = np.ascontiguousarray(x[b].T).astype(bf)
        in_maps.append({"xT": xTb, "mT": M, "wvT": wvT})
    return in_maps


def kernel(x, Wq, Wk, Wv):
    from concourse import bass_utils

    in_maps = prep_inputs(x, Wq, Wk, Wv)
    nc = _get_program(reps=1)
    res = bass_utils.run_bass_kernel_spmd(nc, in_maps, list(range(NCORES)))
    return np.stack([res.results[c]["out"].astype(np.float32)
                     for c in range(NCORES)], axis=0)

